# revision 3
# baseline (speedup 1.0000x reference)
"""Causal self-attention TRN2 Bass kernel.

Problem: B=4, S=2048, D=1024, H=16 heads, head_dim=64, fp32.
Sharding (8 cores): core c handles batch b = c//2 and head-half hg = c%2
(heads hg*8 .. hg*8+7, i.e. columns hg*512..+512 of Wq/Wk/Wv and rows
hg*512..+512 of Wo).  Each core produces a partial [S, D] output; the host
sums the two head-half partials per batch and adds bo.

On-device pipeline per core (all matmuls in float32r: fp32 rounded to 11
mantissa bits, full PE rate at N>=256; inputs pre-rounded on host so the
matmuls are exact w.r.t. the rounded operands):
  phase 1a: V = (x @ Wv + bv) -> DRAM bounce (frees SBUF for phase 1b)
  phase 1b: QT/KT pair tiles [128, S] = (Wq/k_pair_cols.T @ x.T + b)
  phase 2:  per (head, q-chunk of 512): scoresT[k,q] chunks via PE,
            additive causal mask (DVE), exp (ACT, scale=1/8) -> E^T fp32r,
            PV accumulation with a ones-column-augmented V giving both
            attn_outT [64, q] and the softmax row sums in one matmul chain,
            then normalize by reciprocal row sums (DVE) into pair-stacked
            outT tiles (odd heads shifted to partitions 64..127 via DMA).
  phase 3:  out_partial[s, :] = outT_pairs.T @ Wo_rows, DMA to DRAM.
"""

import numpy as np
from contextlib import ExitStack

import concourse.bass as bass
import concourse.tile as tile
import concourse.mybir as mybir
from concourse import bacc
from concourse.bass_utils import run_bass_kernel_spmd

F32 = mybir.dt.float32
F32R = mybir.dt.float32r
ActFn = mybir.ActivationFunctionType

B, S, D = 4, 2048, 1024
H, HD = 16, 64
NCORES = 8
HG = 512           # Wq/Wk/Wv columns (and Wo rows) per core
HPC = 8            # heads per core
P = 128
NDIN = D // P      # 8 contraction chunks for projections
NSC4 = S // 512    # 4 s-chunks of 512
NSC1 = S // P      # 16 s-chunks of 128
NPAIR = 4          # head pairs per core
NEG = -1.0e30


def _build_kernel(ctx: ExitStack, tc: tile.TileContext, io: dict):
    nc = tc.nc
    xt, wq, wk, wv, wo = io["xt"], io["wq"], io["wk"], io["wv"], io["wo"]
    bq, bk, bv, masks, out = io["bq"], io["bk"], io["bv"], io["masks"], io["out"]

    xt_r = xt.rearrange("(a p) s -> a p s", p=P)    # [8, 128, 2048]
    wq_r = wq.rearrange("(a p) n -> a p n", p=P)    # [8, 128, 512]
    wk_r = wk.rearrange("(a p) n -> a p n", p=P)
    wv_r = wv.rearrange("(a p) n -> a p n", p=P)
    wo_r = wo.rearrange("(a p) n -> a p n", p=P)    # [4, 128, 1024]
    bq_r = bq.rearrange("(pr p one) -> pr p one", p=P, one=1)  # [4, 128, 1]
    bk_r = bk.rearrange("(pr p one) -> pr p one", p=P, one=1)

    # ---- persistent pools (whole kernel) ----
    persist = ctx.enter_context(tc.tile_pool(name="persist", bufs=1))
    dram = ctx.enter_context(tc.tile_pool(name="dram", bufs=1, space="DRAM"))

    qt_sb = [persist.tile([P, S], F32R, tag=f"qt{p_}", name=f"qt{p_}")
             for p_ in range(NPAIR)]
    kt_sb = [persist.tile([P, S], F32R, tag=f"kt{p_}", name=f"kt{p_}")
             for p_ in range(NPAIR)]
    bq_sb = [persist.tile([P, 1], F32, tag=f"bq{p_}", name=f"bq{p_}")
             for p_ in range(NPAIR)]
    bk_sb = [persist.tile([P, 1], F32, tag=f"bk{p_}", name=f"bk{p_}")
             for p_ in range(NPAIR)]
    bv_sb = persist.tile([P, HG], F32, tag="bv", name="bv_sb")

    vb = dram.tile([NSC1, P, HG], F32R, tag="vb", name="vb")
    sums_d = dram.tile([HPC, NSC4, 512], F32, tag="sums", name="sums_d")

    for p_ in range(NPAIR):
        nc.sync.dma_start(out=bq_sb[p_][:], in_=bq_r[p_])
        nc.sync.dma_start(out=bk_sb[p_][:], in_=bk_r[p_])
    # broadcast bv [512] across 128 partitions
    bv_b = bass.AP(tensor=bv.tensor, offset=bv.offset,
                   ap=[[0, P]] + [list(a) for a in bv.ap])
    nc.gpsimd.dma_start(out=bv_sb[:], in_=bv_b)

    # ---- phase 1: projections ----
    with tc.tile_pool(name="p1", bufs=1) as p1, \
         tc.tile_pool(name="p1w", bufs=8) as p1w, \
         tc.tile_pool(name="p1tmp", bufs=3) as p1tmp, \
         tc.tile_pool(name="ps1", bufs=4, space="PSUM") as ps1:

        xt_sb = [p1.tile([P, S], F32R, tag=f"xt{a}", name=f"xt{a}")
                 for a in range(NDIN)]
        for a in range(NDIN):
            nc.sync.dma_start(out=xt_sb[a][:], in_=xt_r[a])

        # --- 1a: V -> DRAM bounce ---
        wv_sb = [p1w.tile([P, HG], F32R, tag="w", name=f"wv{a}")
                 for a in range(NDIN)]
        for a in range(NDIN):
            nc.sync.dma_start(out=wv_sb[a][:], in_=wv_r[a])
        for sc in range(NSC1):
            ps = ps1.tile([P, HG], F32, tag="ps", name=f"vps{sc}")
            for a in range(NDIN):
                nc.tensor.matmul(ps[:], xt_sb[a][:, sc * P:(sc + 1) * P],
                                 wv_sb[a][:], start=(a == 0), stop=(a == NDIN - 1))
            vtmp = p1tmp.tile([P, HG], F32R, tag="vtmp", name=f"vtmp{sc}")
            nc.vector.tensor_add(vtmp[:], ps[:], bv_sb[:])
            nc.sync.dma_start(out=vb[sc], in_=vtmp[:])

        # --- 1b: QT / KT pair tiles ---
        for (w_r, b_sb, dst) in ((wq_r, bq_sb, qt_sb), (wk_r, bk_sb, kt_sb)):
            w_sb = [p1w.tile([P, HG], F32R, tag="w", name=f"w{a}")
                    for a in range(NDIN)]
            for a in range(NDIN):
                nc.sync.dma_start(out=w_sb[a][:], in_=w_r[a])
            for p_ in range(NPAIR):
                for sc in range(NSC4):
                    ps = ps1.tile([P, 512], F32, tag="ps", name=f"qkps{p_}_{sc}")
                    for a in range(NDIN):
                        nc.tensor.matmul(
                            ps[:], w_sb[a][:, p_ * P:(p_ + 1) * P],
                            xt_sb[a][:, sc * 512:(sc + 1) * 512],
                            start=(a == 0), stop=(a == NDIN - 1))
                    nc.scalar.activation(dst[p_][:, sc * 512:(sc + 1) * 512],
                                         ps[:], ActFn.Identity, bias=b_sb[p_][:])

    # load masks into SBUF [128, 4, 512]
    mask_t = persist.tile([P, 4, 512], F32, tag="maskt", name="mask_t")
    for v in range(4):
        nc.sync.dma_start(out=mask_t[:, v, :], in_=masks[v])

    # ---- phase 2: attention ----
    with tc.tile_pool(name="p23", bufs=1) as p23:
        v_sb = [p23.tile([P, HPC, 65], F32R, tag=f"v{kb}", name=f"v{kb}")
                for kb in range(NSC1)]
        for kb in range(NSC1):
            nc.sync.dma_start(
                out=v_sb[kb][:, :, 0:64],
                in_=vb[kb].rearrange("p (h d) -> p h d", h=HPC))
            nc.vector.memset(v_sb[kb][:, :, 64:65].bitcast(F32), 1.0)
        outt_sb = [p23.tile([P, S], F32R, tag=f"ot{p_}", name=f"outt{p_}")
                   for p_ in range(NPAIR)]

        with tc.tile_pool(name="p2e", bufs=6) as p2e, \
             tc.tile_pool(name="p2r", bufs=3) as p2r, \
             tc.tile_pool(name="ps2s", bufs=3, space="PSUM") as ps2s, \
             tc.tile_pool(name="ps2o", bufs=2, space="PSUM") as ps2o:

            for h in range(HPC):
                pair, hoff = h // 2, (h % 2) * 64
                for qc in range(NSC4):
                    nkb = 4 * qc + 4
                    ot_ps = ps2o.tile([65, 512], F32, tag="ot",
                                      name=f"otps{h}_{qc}")
                    for kb in range(nkb):
                        sc_ps = ps2s.tile([P, 512], F32, tag="sc",
                                          name=f"scps{h}_{qc}_{kb}")
                        nc.tensor.matmul(
                            sc_ps[:],
                            kt_sb[pair][hoff:hoff + 64, kb * P:(kb + 1) * P],
                            qt_sb[pair][hoff:hoff + 64, qc * 512:(qc + 1) * 512],
                            start=True, stop=True)
                        if kb >= 4 * qc:
                            nc.vector.tensor_add(sc_ps[:], sc_ps[:],
                                                 mask_t[:, kb - 4 * qc, :])
                        e_sb = p2e.tile([P, 512], F32R, tag="e",
                                        name=f"e{h}_{qc}_{kb}")
                        nc.scalar.activation(e_sb[:], sc_ps[:], ActFn.Exp,
                                             scale=0.125)
                        nc.tensor.matmul(ot_ps[:], v_sb[kb][:, h, 0:65],
                                         e_sb[:], start=(kb == 0),
                                         stop=(kb == nkb - 1))
                    # normalize: reciprocal of row sums (partition 64 of ot_ps)
                    rcp = p2r.tile([P, 512], F32, tag="rcp",
                                   name=f"rcp{h}_{qc}")
                    nc.vector.reciprocal(rcp[64:65, :], ot_ps[64:65, :])
                    nc.sync.dma_start(out=sums_d[h, qc], in_=rcp[64:65, :])
                    rb = p2r.tile([64, 512], F32, tag="rb", name=f"rb{h}_{qc}")
                    s_ap = sums_d[h, qc]
                    rb_b = bass.AP(tensor=s_ap.tensor, offset=s_ap.offset,
                                   ap=[[0, 64]] + [list(a) for a in s_ap.ap])
                    nc.gpsimd.dma_start(out=rb[:], in_=rb_b)
                    if h % 2 == 0:
                        nc.vector.tensor_mul(
                            outt_sb[pair][0:64, qc * 512:(qc + 1) * 512],
                            ot_ps[0:64, :], rb[:])
                    else:
                        osc = p2r.tile([64, 512], F32R, tag="osc",
                                       name=f"osc{h}_{qc}")
                        nc.vector.tensor_mul(osc[:], ot_ps[0:64, :], rb[:])
                        nc.sync.dma_start(
                            out=outt_sb[pair][64:128, qc * 512:(qc + 1) * 512],
                            in_=osc[:])

        # ---- phase 3: output projection ----
        with tc.tile_pool(name="p3", bufs=1) as p3, \
             tc.tile_pool(name="p3b", bufs=3) as p3b, \
             tc.tile_pool(name="ps3", bufs=4, space="PSUM") as ps3:
            wo_sb = [p3.tile([P, D], F32R, tag=f"wo{a}", name=f"wo{a}")
                     for a in range(NPAIR)]
            for a in range(NPAIR):
                nc.sync.dma_start(out=wo_sb[a][:], in_=wo_r[a])
            for sc in range(NSC1):
                for nh in range(2):
                    ps = ps3.tile([P, 512], F32, tag="ps",
                                  name=f"ops{sc}_{nh}")
                    for a in range(NPAIR):
                        nc.tensor.matmul(
                            ps[:], outt_sb[a][:, sc * P:(sc + 1) * P],
                            wo_sb[a][:, nh * 512:(nh + 1) * 512],
                            start=(a == 0), stop=(a == NPAIR - 1))
                    ob = p3b.tile([P, 512], F32, tag="ob",
                                  name=f"ob{sc}_{nh}")
                    nc.scalar.copy(ob[:], ps[:])
                    nc.sync.dma_start(
                        out=out[sc * P:(sc + 1) * P, nh * 512:(nh + 1) * 512],
                        in_=ob[:])


def build_bass():
    nc = bacc.Bacc()
    io = {
        "xt": nc.dram_tensor("xt", [D, S], F32R, kind="ExternalInput").ap(),
        "wq": nc.dram_tensor("wq", [D, HG], F32R, kind="ExternalInput").ap(),
        "wk": nc.dram_tensor("wk", [D, HG], F32R, kind="ExternalInput").ap(),
        "wv": nc.dram_tensor("wv", [D, HG], F32R, kind="ExternalInput").ap(),
        "wo": nc.dram_tensor("wo", [HG, D], F32R, kind="ExternalInput").ap(),
        "bq": nc.dram_tensor("bq", [HG], F32, kind="ExternalInput").ap(),
        "bk": nc.dram_tensor("bk", [HG], F32, kind="ExternalInput").ap(),
        "bv": nc.dram_tensor("bv", [HG], F32, kind="ExternalInput").ap(),
        "masks": nc.dram_tensor("masks", [4, P, 512], F32,
                                kind="ExternalInput").ap(),
        "out": nc.dram_tensor("out", [S, D], F32, kind="ExternalOutput").ap(),
    }
    with tile.TileContext(nc) as tc, ExitStack() as ctx:
        _build_kernel(ctx, tc, io)
    nc.compile()
    return nc


def round_fp32r(a: np.ndarray) -> np.ndarray:
    """Round fp32 to the 11-mantissa-bit float32r grid (nearest even)."""
    a = np.ascontiguousarray(a, dtype=np.float32)
    u = a.view(np.uint32)
    lm = np.uint32((1 << 12) - 1)
    half = np.uint32(1 << 11)
    low = u & lm
    hi = u & ~lm
    inc = (low > half) | ((low == half) &
                          (((hi >> np.uint32(12)) & np.uint32(1)).astype(bool)))
    return (hi + inc.astype(np.uint32) * np.uint32(1 << 12)).view(np.float32)


_NC_CACHE = {}


def _get_nc():
    if "nc" not in _NC_CACHE:
        _NC_CACHE["nc"] = build_bass()
    return _NC_CACHE["nc"]


def _make_masks() -> np.ndarray:
    k = np.arange(P)[:, None]
    c = np.arange(512)[None, :]
    m = np.zeros((4, P, 512), dtype=np.float32)
    for v in range(4):
        m[v] = np.where((c - 128 * v) >= k, 0.0, NEG)
    return m


def kernel(x, Wq, bq, Wk, bk, Wv, bv, Wo, bo, _trace=False):
    nc = _get_nc()
    x = np.asarray(x, dtype=np.float32)
    Wq, Wk, Wv, Wo = (np.asarray(w, dtype=np.float32) for w in (Wq, Wk, Wv, Wo))
    bq, bk, bv, bo = (np.asarray(b_, dtype=np.float32) for b_ in (bq, bk, bv, bo))
    masks = _make_masks()

    in_maps = []
    for c in range(NCORES):
        b_, hg = c // 2, c % 2
        cols = slice(hg * HG, (hg + 1) * HG)
        in_maps.append({
            "xt": round_fp32r(x[b_].T),
            "wq": round_fp32r(Wq[:, cols]),
            "wk": round_fp32r(Wk[:, cols]),
            "wv": round_fp32r(Wv[:, cols]),
            "wo": round_fp32r(Wo[cols, :]),
            "bq": np.ascontiguousarray(bq[cols]),
            "bk": np.ascontiguousarray(bk[cols]),
            "bv": np.ascontiguousarray(bv[cols]),
            "masks": masks,
        })
    res = run_bass_kernel_spmd(nc, in_maps, list(range(NCORES)), trace=_trace)
    out = np.empty((B, S, D), dtype=np.float32)
    for b_ in range(B):
        out[b_] = res.results[2 * b_]["out"] + res.results[2 * b_ + 1]["out"]
    out += bo[None, None, :]
    if _trace:
        return out, res
    return out


# revision 5
# speedup vs baseline: 1.0441x; 1.0441x over previous
"""Causal self-attention TRN2 Bass kernel.

Problem: B=4, S=2048, D=1024, H=16 heads, head_dim=64, fp32.
Sharding (8 cores): core c handles batch b = c//2 and head-half hg = c%2
(heads hg*8 .. hg*8+7, i.e. columns hg*512..+512 of Wq/Wk/Wv and rows
hg*512..+512 of Wo).  Each core produces a partial [S, D] output; the host
sums the two head-half partials per batch and adds bo.

On-device pipeline per core (all matmuls in float32r: fp32 rounded to 11
mantissa bits, full PE rate at N>=256; inputs pre-rounded on host so the
matmuls are exact w.r.t. the rounded operands):
  phase 1a: V = (x @ Wv + bv) -> DRAM bounce (frees SBUF for phase 1b)
  phase 1b: QT/KT pair tiles [128, S] = (Wq/k_pair_cols.T @ x.T + b)
  phase 2:  per (head, q-chunk of 512): scoresT[k,q] chunks via PE,
            additive causal mask (DVE), exp (ACT, scale=1/8) -> E^T fp32r,
            PV accumulation with a ones-column-augmented V giving both
            attn_outT [64, q] and the softmax row sums in one matmul chain,
            then normalize by reciprocal row sums (DVE) into pair-stacked
            outT tiles (odd heads shifted to partitions 64..127 via DMA).
  phase 3:  out_partial[s, :] = outT_pairs.T @ Wo_rows, DMA to DRAM.
"""

import numpy as np
from contextlib import ExitStack

import concourse.bass as bass
import concourse.tile as tile
import concourse.mybir as mybir
from concourse import bacc
from concourse.bass_utils import run_bass_kernel_spmd

F32 = mybir.dt.float32
F32R = mybir.dt.float32r
ActFn = mybir.ActivationFunctionType

B, S, D = 4, 2048, 1024
H, HD = 16, 64
NCORES = 8
HG = 512           # Wq/Wk/Wv columns (and Wo rows) per core
HPC = 8            # heads per core
P = 128
NDIN = D // P      # 8 contraction chunks for projections
NSC4 = S // 512    # 4 s-chunks of 512
NSC1 = S // P      # 16 s-chunks of 128
NPAIR = 4          # head pairs per core
NEG = -1.0e30


def _build_kernel(ctx: ExitStack, tc: tile.TileContext, io: dict):
    nc = tc.nc
    xt, wq, wk, wv, wo = io["xt"], io["wq"], io["wk"], io["wv"], io["wo"]
    bq, bk, bv, masks, out = io["bq"], io["bk"], io["bv"], io["masks"], io["out"]

    xt_r = xt.rearrange("(a p) s -> a p s", p=P)    # [8, 128, 2048]
    wq_r = wq.rearrange("(a p) n -> a p n", p=P)    # [8, 128, 512]
    wk_r = wk.rearrange("(a p) n -> a p n", p=P)
    wv_r = wv.rearrange("(a p) n -> a p n", p=P)
    wo_r = wo.rearrange("(a p) n -> a p n", p=P)    # [4, 128, 1024]
    bq_r = bq.rearrange("(pr p one) -> pr p one", p=P, one=1)  # [4, 128, 1]
    bk_r = bk.rearrange("(pr p one) -> pr p one", p=P, one=1)

    # ---- persistent pools (whole kernel) ----
    persist = ctx.enter_context(tc.tile_pool(name="persist", bufs=1))
    dram = ctx.enter_context(tc.tile_pool(name="dram", bufs=1, space="DRAM"))

    qt_sb = [persist.tile([P, S], F32R, tag=f"qt{p_}", name=f"qt{p_}")
             for p_ in range(NPAIR)]
    kt_sb = [persist.tile([P, S], F32R, tag=f"kt{p_}", name=f"kt{p_}")
             for p_ in range(NPAIR)]
    bq_sb = [persist.tile([P, 1], F32, tag=f"bq{p_}", name=f"bq{p_}")
             for p_ in range(NPAIR)]
    bk_sb = [persist.tile([P, 1], F32, tag=f"bk{p_}", name=f"bk{p_}")
             for p_ in range(NPAIR)]
    bv_sb = persist.tile([P, HG], F32, tag="bv", name="bv_sb")

    vb = dram.tile([NSC1, P, HG], F32R, tag="vb", name="vb")
    sums_d = dram.tile([HPC, NSC4, 512], F32, tag="sums", name="sums_d")

    for p_ in range(NPAIR):
        nc.sync.dma_start(out=bq_sb[p_][:], in_=bq_r[p_])
        nc.sync.dma_start(out=bk_sb[p_][:], in_=bk_r[p_])
    # broadcast bv [512] across 128 partitions
    bv_b = bass.AP(tensor=bv.tensor, offset=bv.offset,
                   ap=[[0, P]] + [list(a) for a in bv.ap])
    nc.gpsimd.dma_start(out=bv_sb[:], in_=bv_b)

    # ---- phase 1: projections ----
    with tc.tile_pool(name="p1", bufs=1) as p1, \
         tc.tile_pool(name="p1w", bufs=8) as p1w, \
         tc.tile_pool(name="p1tmp", bufs=3) as p1tmp, \
         tc.tile_pool(name="ps1", bufs=4, space="PSUM") as ps1:

        xt_sb = [p1.tile([P, S], F32R, tag=f"xt{a}", name=f"xt{a}")
                 for a in range(NDIN)]
        for a in range(NDIN):
            nc.sync.dma_start(out=xt_sb[a][:], in_=xt_r[a])

        # --- 1a: V -> DRAM bounce ---
        wv_sb = [p1w.tile([P, HG], F32R, tag="w", name=f"wv{a}")
                 for a in range(NDIN)]
        for a in range(NDIN):
            nc.sync.dma_start(out=wv_sb[a][:], in_=wv_r[a])
        for sc in range(NSC1):
            ps = ps1.tile([P, HG], F32, tag="ps", name=f"vps{sc}")
            for a in range(NDIN):
                nc.tensor.matmul(ps[:], xt_sb[a][:, sc * P:(sc + 1) * P],
                                 wv_sb[a][:], start=(a == 0), stop=(a == NDIN - 1))
            vtmp = p1tmp.tile([P, HG], F32R, tag="vtmp", name=f"vtmp{sc}")
            nc.vector.tensor_add(vtmp[:], ps[:], bv_sb[:])
            nc.sync.dma_start(out=vb[sc], in_=vtmp[:])

        # --- 1b: QT / KT pair tiles ---
        for (w_r, b_sb, dst) in ((wq_r, bq_sb, qt_sb), (wk_r, bk_sb, kt_sb)):
            w_sb = [p1w.tile([P, HG], F32R, tag="w", name=f"w{a}")
                    for a in range(NDIN)]
            for a in range(NDIN):
                nc.sync.dma_start(out=w_sb[a][:], in_=w_r[a])
            for p_ in range(NPAIR):
                for sc in range(NSC4):
                    ps = ps1.tile([P, 512], F32, tag="ps", name=f"qkps{p_}_{sc}")
                    for a in range(NDIN):
                        nc.tensor.matmul(
                            ps[:], w_sb[a][:, p_ * P:(p_ + 1) * P],
                            xt_sb[a][:, sc * 512:(sc + 1) * 512],
                            start=(a == 0), stop=(a == NDIN - 1))
                    nc.scalar.activation(dst[p_][:, sc * 512:(sc + 1) * 512],
                                         ps[:], ActFn.Identity, bias=b_sb[p_][:])

    # load masks into SBUF [128, 4, 512]
    mask_t = persist.tile([P, 4, 512], F32, tag="maskt", name="mask_t")
    for v in range(4):
        nc.sync.dma_start(out=mask_t[:, v, :], in_=masks[v])

    # ---- phase 2: attention ----
    with tc.tile_pool(name="p23", bufs=1) as p23:
        v_sb = [p23.tile([P, HPC, 65], F32R, tag=f"v{kb}", name=f"v{kb}")
                for kb in range(NSC1)]
        for kb in range(NSC1):
            nc.sync.dma_start(
                out=v_sb[kb][:, :, 0:64],
                in_=vb[kb].rearrange("p (h d) -> p h d", h=HPC))
            nc.vector.memset(v_sb[kb][:, :, 64:65].bitcast(F32), 1.0)
        outt_sb = [p23.tile([P, S], F32R, tag=f"ot{p_}", name=f"outt{p_}")
                   for p_ in range(NPAIR)]

        with tc.tile_pool(name="p2e", bufs=6) as p2e, \
             tc.tile_pool(name="p2r", bufs=3) as p2r, \
             tc.tile_pool(name="ps2s", bufs=3, space="PSUM") as ps2s, \
             tc.tile_pool(name="ps2o", bufs=2, space="PSUM") as ps2o:

            for h in range(HPC):
                pair, hoff = h // 2, (h % 2) * 64
                for qc in range(NSC4):
                    nkb = 4 * qc + 4
                    ot_ps = ps2o.tile([65, 512], F32, tag="ot",
                                      name=f"otps{h}_{qc}")
                    for kb in range(nkb):
                        # causal trim: diag-block chunks only need columns
                        # q >= kb*128, i.e. [off:512] of this 512-wide chunk
                        off = max(0, (kb - 4 * qc) * P)
                        nw = 512 - off
                        sc_ps = ps2s.tile([P, 512], F32, tag="sc",
                                          name=f"scps{h}_{qc}_{kb}")
                        nc.tensor.matmul(
                            sc_ps[:, 0:nw],
                            kt_sb[pair][hoff:hoff + 64, kb * P:(kb + 1) * P],
                            qt_sb[pair][hoff:hoff + 64,
                                        qc * 512 + off:(qc + 1) * 512],
                            start=True, stop=True)
                        if kb >= 4 * qc:
                            # triangular mask on the leading 128 cols (q == k)
                            nc.vector.tensor_add(sc_ps[:, 0:P], sc_ps[:, 0:P],
                                                 mask_t[:, 0, 0:P])
                        e_sb = p2e.tile([P, 512], F32R, tag="e",
                                        name=f"e{h}_{qc}_{kb}")
                        nc.scalar.activation(e_sb[:, 0:nw], sc_ps[:, 0:nw],
                                             ActFn.Exp, scale=0.125)
                        nc.tensor.matmul(ot_ps[:, off:512],
                                         v_sb[kb][:, h, 0:65],
                                         e_sb[:, 0:nw], start=(kb == 0),
                                         stop=(kb == nkb - 1),
                                         skip_group_check=True)
                    # normalize: reciprocal of row sums (partition 64 of ot_ps)
                    rcp = p2r.tile([P, 512], F32, tag="rcp",
                                   name=f"rcp{h}_{qc}")
                    nc.vector.reciprocal(rcp[64:65, :], ot_ps[64:65, :])
                    nc.sync.dma_start(out=sums_d[h, qc], in_=rcp[64:65, :])
                    rb = p2r.tile([64, 512], F32, tag="rb", name=f"rb{h}_{qc}")
                    s_ap = sums_d[h, qc]
                    rb_b = bass.AP(tensor=s_ap.tensor, offset=s_ap.offset,
                                   ap=[[0, 64]] + [list(a) for a in s_ap.ap])
                    nc.gpsimd.dma_start(out=rb[:], in_=rb_b)
                    if h % 2 == 0:
                        nc.vector.tensor_mul(
                            outt_sb[pair][0:64, qc * 512:(qc + 1) * 512],
                            ot_ps[0:64, :], rb[:])
                    else:
                        osc = p2r.tile([64, 512], F32R, tag="osc",
                                       name=f"osc{h}_{qc}")
                        nc.vector.tensor_mul(osc[:], ot_ps[0:64, :], rb[:])
                        nc.sync.dma_start(
                            out=outt_sb[pair][64:128, qc * 512:(qc + 1) * 512],
                            in_=osc[:])

        # ---- phase 3: output projection ----
        with tc.tile_pool(name="p3", bufs=1) as p3, \
             tc.tile_pool(name="p3b", bufs=3) as p3b, \
             tc.tile_pool(name="ps3", bufs=4, space="PSUM") as ps3:
            wo_sb = [p3.tile([P, D], F32R, tag=f"wo{a}", name=f"wo{a}")
                     for a in range(NPAIR)]
            for a in range(NPAIR):
                nc.sync.dma_start(out=wo_sb[a][:], in_=wo_r[a])
            for sc in range(NSC1):
                for nh in range(2):
                    ps = ps3.tile([P, 512], F32, tag="ps",
                                  name=f"ops{sc}_{nh}")
                    for a in range(NPAIR):
                        nc.tensor.matmul(
                            ps[:], outt_sb[a][:, sc * P:(sc + 1) * P],
                            wo_sb[a][:, nh * 512:(nh + 1) * 512],
                            start=(a == 0), stop=(a == NPAIR - 1))
                    ob = p3b.tile([P, 512], F32, tag="ob",
                                  name=f"ob{sc}_{nh}")
                    nc.scalar.copy(ob[:], ps[:])
                    nc.sync.dma_start(
                        out=out[sc * P:(sc + 1) * P, nh * 512:(nh + 1) * 512],
                        in_=ob[:])


def build_bass():
    nc = bacc.Bacc()
    io = {
        "xt": nc.dram_tensor("xt", [D, S], F32R, kind="ExternalInput").ap(),
        "wq": nc.dram_tensor("wq", [D, HG], F32R, kind="ExternalInput").ap(),
        "wk": nc.dram_tensor("wk", [D, HG], F32R, kind="ExternalInput").ap(),
        "wv": nc.dram_tensor("wv", [D, HG], F32R, kind="ExternalInput").ap(),
        "wo": nc.dram_tensor("wo", [HG, D], F32R, kind="ExternalInput").ap(),
        "bq": nc.dram_tensor("bq", [HG], F32, kind="ExternalInput").ap(),
        "bk": nc.dram_tensor("bk", [HG], F32, kind="ExternalInput").ap(),
        "bv": nc.dram_tensor("bv", [HG], F32, kind="ExternalInput").ap(),
        "masks": nc.dram_tensor("masks", [4, P, 512], F32,
                                kind="ExternalInput").ap(),
        "out": nc.dram_tensor("out", [S, D], F32, kind="ExternalOutput").ap(),
    }
    with tile.TileContext(nc) as tc, ExitStack() as ctx:
        _build_kernel(ctx, tc, io)
    nc.compile()
    return nc


def round_fp32r(a: np.ndarray) -> np.ndarray:
    """Round fp32 to the 11-mantissa-bit float32r grid (nearest even)."""
    a = np.ascontiguousarray(a, dtype=np.float32)
    u = a.view(np.uint32)
    lm = np.uint32((1 << 12) - 1)
    half = np.uint32(1 << 11)
    low = u & lm
    hi = u & ~lm
    inc = (low > half) | ((low == half) &
                          (((hi >> np.uint32(12)) & np.uint32(1)).astype(bool)))
    return (hi + inc.astype(np.uint32) * np.uint32(1 << 12)).view(np.float32)


_NC_CACHE = {}


def _get_nc():
    if "nc" not in _NC_CACHE:
        _NC_CACHE["nc"] = build_bass()
    return _NC_CACHE["nc"]


def _make_masks() -> np.ndarray:
    k = np.arange(P)[:, None]
    c = np.arange(512)[None, :]
    m = np.zeros((4, P, 512), dtype=np.float32)
    for v in range(4):
        m[v] = np.where((c - 128 * v) >= k, 0.0, NEG)
    return m


def kernel(x, Wq, bq, Wk, bk, Wv, bv, Wo, bo, _trace=False):
    nc = _get_nc()
    x = np.asarray(x, dtype=np.float32)
    Wq, Wk, Wv, Wo = (np.asarray(w, dtype=np.float32) for w in (Wq, Wk, Wv, Wo))
    bq, bk, bv, bo = (np.asarray(b_, dtype=np.float32) for b_ in (bq, bk, bv, bo))
    masks = _make_masks()

    in_maps = []
    for c in range(NCORES):
        b_, hg = c // 2, c % 2
        cols = slice(hg * HG, (hg + 1) * HG)
        in_maps.append({
            "xt": round_fp32r(x[b_].T),
            "wq": round_fp32r(Wq[:, cols]),
            "wk": round_fp32r(Wk[:, cols]),
            "wv": round_fp32r(Wv[:, cols]),
            "wo": round_fp32r(Wo[cols, :]),
            "bq": np.ascontiguousarray(bq[cols]),
            "bk": np.ascontiguousarray(bk[cols]),
            "bv": np.ascontiguousarray(bv[cols]),
            "masks": masks,
        })
    res = run_bass_kernel_spmd(nc, in_maps, list(range(NCORES)), trace=_trace)
    out = np.empty((B, S, D), dtype=np.float32)
    for b_ in range(B):
        out[b_] = res.results[2 * b_]["out"] + res.results[2 * b_ + 1]["out"]
    out += bo[None, None, :]
    if _trace:
        return out, res
    return out


# revision 7
# speedup vs baseline: 1.1065x; 1.0597x over previous
"""Causal self-attention TRN2 Bass kernel.

Problem: B=4, S=2048, D=1024, H=16 heads, head_dim=64, fp32.
Sharding (8 cores): core c handles batch b = c//2 and head-half hg = c%2
(heads hg*8 .. hg*8+7, i.e. columns hg*512..+512 of Wq/Wk/Wv and rows
hg*512..+512 of Wo).  Each core produces a partial [S, D] output; the host
sums the two head-half partials per batch and adds bo.

On-device pipeline per core (all matmuls in float32r: fp32 rounded to 11
mantissa bits, full PE rate at N>=256; inputs pre-rounded on host so the
matmuls are exact w.r.t. the rounded operands):
  phase 1a: V = (x @ Wv + bv) -> DRAM bounce (frees SBUF for phase 1b)
  phase 1b: QT/KT pair tiles [128, S] = (Wq/k_pair_cols.T @ x.T + b)
  phase 2:  per (head, q-chunk of 512): scoresT[k,q] chunks via PE,
            additive causal mask (DVE), exp (ACT, scale=1/8) -> E^T fp32r,
            PV accumulation with a ones-column-augmented V giving both
            attn_outT [64, q] and the softmax row sums in one matmul chain,
            then normalize by reciprocal row sums (DVE) into pair-stacked
            outT tiles (odd heads shifted to partitions 64..127 via DMA).
  phase 3:  out_partial[s, :] = outT_pairs.T @ Wo_rows, DMA to DRAM.
"""

import numpy as np
from contextlib import ExitStack

import concourse.bass as bass
import concourse.tile as tile
import concourse.mybir as mybir
from concourse import bacc
from concourse.bass_utils import run_bass_kernel_spmd

F32 = mybir.dt.float32
F32R = mybir.dt.float32r
ActFn = mybir.ActivationFunctionType

B, S, D = 4, 2048, 1024
H, HD = 16, 64
NCORES = 8
HG = 512           # Wq/Wk/Wv columns (and Wo rows) per core
HPC = 8            # heads per core
P = 128
NDIN = D // P      # 8 contraction chunks for projections
NSC4 = S // 512    # 4 s-chunks of 512
NSC1 = S // P      # 16 s-chunks of 128
NPAIR = 4          # head pairs per core
NEG = -1.0e30


def _build_kernel(ctx: ExitStack, tc: tile.TileContext, io: dict):
    nc = tc.nc
    xt, wq, wk, wv, wo = io["xt"], io["wq"], io["wk"], io["wv"], io["wo"]
    bq, bk, bv, masks, out = io["bq"], io["bk"], io["bv"], io["masks"], io["out"]

    xt_r = xt.rearrange("(a p) s -> a p s", p=P)    # [8, 128, 2048]
    wq_r = wq.rearrange("(a p) n -> a p n", p=P)    # [8, 128, 512]
    wk_r = wk.rearrange("(a p) n -> a p n", p=P)
    wv_r = wv.rearrange("(a p) n -> a p n", p=P)
    wo_r = wo.rearrange("(a p) n -> a p n", p=P)    # [4, 128, 1024]
    bq_r = bq.rearrange("(pr p one) -> pr p one", p=P, one=1)  # [4, 128, 1]
    bk_r = bk.rearrange("(pr p one) -> pr p one", p=P, one=1)

    # ---- persistent pools (whole kernel) ----
    persist = ctx.enter_context(tc.tile_pool(name="persist", bufs=1))
    dram = ctx.enter_context(tc.tile_pool(name="dram", bufs=1, space="DRAM"))

    qt_sb = [persist.tile([P, S], F32R, tag=f"qt{p_}", name=f"qt{p_}")
             for p_ in range(NPAIR)]
    kt_sb = [persist.tile([P, S], F32R, tag=f"kt{p_}", name=f"kt{p_}")
             for p_ in range(NPAIR)]
    bq_sb = [persist.tile([P, 1], F32, tag=f"bq{p_}", name=f"bq{p_}")
             for p_ in range(NPAIR)]
    bk_sb = [persist.tile([P, 1], F32, tag=f"bk{p_}", name=f"bk{p_}")
             for p_ in range(NPAIR)]
    bv_sb = persist.tile([P, HG], F32, tag="bv", name="bv_sb")

    vb = dram.tile([NSC1, P, HG], F32R, tag="vb", name="vb")
    sums_d = dram.tile([HPC, NSC4, 512], F32, tag="sums", name="sums_d")

    for p_ in range(NPAIR):
        nc.sync.dma_start(out=bq_sb[p_][:], in_=bq_r[p_])
        nc.sync.dma_start(out=bk_sb[p_][:], in_=bk_r[p_])
    # broadcast bv [512] across 128 partitions
    bv_b = bass.AP(tensor=bv.tensor, offset=bv.offset,
                   ap=[[0, P]] + [list(a) for a in bv.ap])
    nc.gpsimd.dma_start(out=bv_sb[:], in_=bv_b)

    # ---- phase 1: projections ----
    with tc.tile_pool(name="p1", bufs=1) as p1, \
         tc.tile_pool(name="p1w", bufs=8) as p1w, \
         tc.tile_pool(name="p1tmp", bufs=3) as p1tmp, \
         tc.tile_pool(name="ps1", bufs=4, space="PSUM") as ps1:

        xt_sb = [p1.tile([P, S], F32R, tag=f"xt{a}", name=f"xt{a}")
                 for a in range(NDIN)]
        for a in range(NDIN):
            nc.sync.dma_start(out=xt_sb[a][:], in_=xt_r[a])

        # --- 1a: V -> DRAM bounce ---
        wv_sb = [p1w.tile([P, HG], F32R, tag="w", name=f"wv{a}")
                 for a in range(NDIN)]
        for a in range(NDIN):
            nc.sync.dma_start(out=wv_sb[a][:], in_=wv_r[a])
        for sc in range(NSC1):
            ps = ps1.tile([P, HG], F32, tag="ps", name=f"vps{sc}")
            for a in range(NDIN):
                nc.tensor.matmul(ps[:], xt_sb[a][:, sc * P:(sc + 1) * P],
                                 wv_sb[a][:], start=(a == 0), stop=(a == NDIN - 1))
            vtmp = p1tmp.tile([P, HG], F32R, tag="vtmp", name=f"vtmp{sc}")
            nc.vector.tensor_add(vtmp[:], ps[:], bv_sb[:])
            nc.sync.dma_start(out=vb[sc], in_=vtmp[:])

        # --- 1b: QT / KT pair tiles ---
        for (w_r, b_sb, dst) in ((wq_r, bq_sb, qt_sb), (wk_r, bk_sb, kt_sb)):
            w_sb = [p1w.tile([P, HG], F32R, tag="w", name=f"w{a}")
                    for a in range(NDIN)]
            for a in range(NDIN):
                nc.sync.dma_start(out=w_sb[a][:], in_=w_r[a])
            for p_ in range(NPAIR):
                for sc in range(NSC4):
                    ps = ps1.tile([P, 512], F32, tag="ps", name=f"qkps{p_}_{sc}")
                    for a in range(NDIN):
                        nc.tensor.matmul(
                            ps[:], w_sb[a][:, p_ * P:(p_ + 1) * P],
                            xt_sb[a][:, sc * 512:(sc + 1) * 512],
                            start=(a == 0), stop=(a == NDIN - 1))
                    nc.scalar.activation(dst[p_][:, sc * 512:(sc + 1) * 512],
                                         ps[:], ActFn.Identity, bias=b_sb[p_][:])

    # load masks into SBUF [128, 4, 512]
    mask_t = persist.tile([P, 4, 512], F32, tag="maskt", name="mask_t")
    for v in range(4):
        nc.sync.dma_start(out=mask_t[:, v, :], in_=masks[v])

    # ---- phase 2: attention ----
    with tc.tile_pool(name="p23", bufs=1) as p23:
        v_sb = [p23.tile([P, HPC, 65], F32R, tag=f"v{kb}", name=f"v{kb}")
                for kb in range(NSC1)]
        for kb in range(NSC1):
            nc.sync.dma_start(
                out=v_sb[kb][:, :, 0:64],
                in_=vb[kb].rearrange("p (h d) -> p h d", h=HPC))
            nc.vector.memset(v_sb[kb][:, :, 64:65].bitcast(F32), 1.0)
        outt_sb = [p23.tile([P, S], F32R, tag=f"ot{p_}", name=f"outt{p_}")
                   for p_ in range(NPAIR)]

        with tc.tile_pool(name="p2e", bufs=6) as p2e, \
             tc.tile_pool(name="p2r", bufs=3) as p2r, \
             tc.tile_pool(name="ps2s", bufs=3, space="PSUM") as ps2s, \
             tc.tile_pool(name="ps2o", bufs=2, space="PSUM") as ps2o:

            for h in range(HPC):
                pair, hoff = h // 2, (h % 2) * 64
                for qc in range(NSC4):
                    nkb = 4 * qc + 4
                    ot_ps = ps2o.tile([65, 512], F32, tag="ot",
                                      name=f"otps{h}_{qc}")
                    for kb in range(nkb):
                        # causal trim: diag-block chunks only need columns
                        # q >= kb*128, i.e. [off:512] of this 512-wide chunk
                        off = max(0, (kb - 4 * qc) * P)
                        nw = 512 - off
                        sc_ps = ps2s.tile([P, 512], F32, tag="sc",
                                          name=f"scps{h}_{qc}_{kb}")
                        nc.tensor.matmul(
                            sc_ps[:, 0:nw],
                            kt_sb[pair][hoff:hoff + 64, kb * P:(kb + 1) * P],
                            qt_sb[pair][hoff:hoff + 64,
                                        qc * 512 + off:(qc + 1) * 512],
                            start=True, stop=True)
                        if kb >= 4 * qc:
                            # triangular mask on the leading 128 cols (q == k)
                            nc.vector.tensor_add(sc_ps[:, 0:P], sc_ps[:, 0:P],
                                                 mask_t[:, 0, 0:P])
                        e_sb = p2e.tile([P, 512], F32R, tag="e",
                                        name=f"e{h}_{qc}_{kb}")
                        nc.scalar.activation(e_sb[:, 0:nw], sc_ps[:, 0:nw],
                                             ActFn.Exp, scale=0.125)
                        nc.tensor.matmul(ot_ps[:, off:512],
                                         v_sb[kb][:, h, 0:65],
                                         e_sb[:, 0:nw], start=(kb == 0),
                                         stop=(kb == nkb - 1),
                                         skip_group_check=True)
                    # normalize: broadcast row sums (partition 64 of ot_ps)
                    # via DRAM, then approx-reciprocal + multiply on DVE
                    srow = p2r.tile([P, 512], F32, tag="srow",
                                    name=f"srow{h}_{qc}")
                    nc.scalar.copy(srow[64:65, :], ot_ps[64:65, :])
                    nc.sync.dma_start(out=sums_d[h, qc], in_=srow[64:65, :])
                    sb_ = p2r.tile([64, 512], F32, tag="sb", name=f"sb{h}_{qc}")
                    s_ap = sums_d[h, qc]
                    sb_b = bass.AP(tensor=s_ap.tensor, offset=s_ap.offset,
                                   ap=[[0, 64]] + [list(a) for a in s_ap.ap])
                    nc.gpsimd.dma_start(out=sb_[:], in_=sb_b)
                    rb = p2r.tile([64, 512], F32, tag="rb", name=f"rb{h}_{qc}")
                    nc.vector.reciprocal_approx_fast(rb[:], sb_[:])
                    if h % 2 == 0:
                        nc.vector.tensor_mul(
                            outt_sb[pair][0:64, qc * 512:(qc + 1) * 512],
                            ot_ps[0:64, :], rb[:])
                    else:
                        osc = p2r.tile([64, 512], F32R, tag="osc",
                                       name=f"osc{h}_{qc}")
                        nc.vector.tensor_mul(osc[:], ot_ps[0:64, :], rb[:])
                        nc.sync.dma_start(
                            out=outt_sb[pair][64:128, qc * 512:(qc + 1) * 512],
                            in_=osc[:])

        # ---- phase 3: output projection ----
        with tc.tile_pool(name="p3", bufs=1) as p3, \
             tc.tile_pool(name="p3b", bufs=3) as p3b, \
             tc.tile_pool(name="ps3", bufs=4, space="PSUM") as ps3:
            wo_sb = [p3.tile([P, D], F32R, tag=f"wo{a}", name=f"wo{a}")
                     for a in range(NPAIR)]
            for a in range(NPAIR):
                nc.sync.dma_start(out=wo_sb[a][:], in_=wo_r[a])
            for sc in range(NSC1):
                for nh in range(2):
                    ps = ps3.tile([P, 512], F32, tag="ps",
                                  name=f"ops{sc}_{nh}")
                    for a in range(NPAIR):
                        nc.tensor.matmul(
                            ps[:], outt_sb[a][:, sc * P:(sc + 1) * P],
                            wo_sb[a][:, nh * 512:(nh + 1) * 512],
                            start=(a == 0), stop=(a == NPAIR - 1))
                    ob = p3b.tile([P, 512], F32, tag="ob",
                                  name=f"ob{sc}_{nh}")
                    nc.scalar.copy(ob[:], ps[:])
                    nc.sync.dma_start(
                        out=out[sc * P:(sc + 1) * P, nh * 512:(nh + 1) * 512],
                        in_=ob[:])


def build_bass():
    nc = bacc.Bacc()
    io = {
        "xt": nc.dram_tensor("xt", [D, S], F32R, kind="ExternalInput").ap(),
        "wq": nc.dram_tensor("wq", [D, HG], F32R, kind="ExternalInput").ap(),
        "wk": nc.dram_tensor("wk", [D, HG], F32R, kind="ExternalInput").ap(),
        "wv": nc.dram_tensor("wv", [D, HG], F32R, kind="ExternalInput").ap(),
        "wo": nc.dram_tensor("wo", [HG, D], F32R, kind="ExternalInput").ap(),
        "bq": nc.dram_tensor("bq", [HG], F32, kind="ExternalInput").ap(),
        "bk": nc.dram_tensor("bk", [HG], F32, kind="ExternalInput").ap(),
        "bv": nc.dram_tensor("bv", [HG], F32, kind="ExternalInput").ap(),
        "masks": nc.dram_tensor("masks", [4, P, 512], F32,
                                kind="ExternalInput").ap(),
        "out": nc.dram_tensor("out", [S, D], F32, kind="ExternalOutput").ap(),
    }
    with tile.TileContext(nc) as tc, ExitStack() as ctx:
        _build_kernel(ctx, tc, io)
    nc.compile()
    return nc


def round_fp32r(a: np.ndarray) -> np.ndarray:
    """Round fp32 to the 11-mantissa-bit float32r grid (nearest even)."""
    a = np.ascontiguousarray(a, dtype=np.float32)
    u = a.view(np.uint32)
    lm = np.uint32((1 << 12) - 1)
    half = np.uint32(1 << 11)
    low = u & lm
    hi = u & ~lm
    inc = (low > half) | ((low == half) &
                          (((hi >> np.uint32(12)) & np.uint32(1)).astype(bool)))
    return (hi + inc.astype(np.uint32) * np.uint32(1 << 12)).view(np.float32)


_NC_CACHE = {}


def _get_nc():
    if "nc" not in _NC_CACHE:
        _NC_CACHE["nc"] = build_bass()
    return _NC_CACHE["nc"]


def _make_masks() -> np.ndarray:
    k = np.arange(P)[:, None]
    c = np.arange(512)[None, :]
    m = np.zeros((4, P, 512), dtype=np.float32)
    for v in range(4):
        m[v] = np.where((c - 128 * v) >= k, 0.0, NEG)
    return m


def kernel(x, Wq, bq, Wk, bk, Wv, bv, Wo, bo, _trace=False):
    nc = _get_nc()
    x = np.asarray(x, dtype=np.float32)
    Wq, Wk, Wv, Wo = (np.asarray(w, dtype=np.float32) for w in (Wq, Wk, Wv, Wo))
    bq, bk, bv, bo = (np.asarray(b_, dtype=np.float32) for b_ in (bq, bk, bv, bo))
    masks = _make_masks()

    in_maps = []
    for c in range(NCORES):
        b_, hg = c // 2, c % 2
        cols = slice(hg * HG, (hg + 1) * HG)
        in_maps.append({
            "xt": round_fp32r(x[b_].T),
            "wq": round_fp32r(Wq[:, cols]),
            "wk": round_fp32r(Wk[:, cols]),
            "wv": round_fp32r(Wv[:, cols]),
            "wo": round_fp32r(Wo[cols, :]),
            "bq": np.ascontiguousarray(bq[cols]),
            "bk": np.ascontiguousarray(bk[cols]),
            "bv": np.ascontiguousarray(bv[cols]),
            "masks": masks,
        })
    res = run_bass_kernel_spmd(nc, in_maps, list(range(NCORES)), trace=_trace)
    out = np.empty((B, S, D), dtype=np.float32)
    for b_ in range(B):
        out[b_] = res.results[2 * b_]["out"] + res.results[2 * b_ + 1]["out"]
    out += bo[None, None, :]
    if _trace:
        return out, res
    return out


# revision 8
# speedup vs baseline: 1.2534x; 1.1327x over previous
"""Causal self-attention TRN2 Bass kernel.

Problem: B=4, S=2048, D=1024, H=16 heads, head_dim=64, fp32.
Sharding (8 cores): core c handles batch b = c//2 and head-half hg = c%2
(heads hg*8 .. hg*8+7, i.e. columns hg*512..+512 of Wq/Wk/Wv and rows
hg*512..+512 of Wo).  Each core produces a partial [S, D] output; the host
sums the two head-half partials per batch and adds bo.

On-device pipeline per core (all matmuls in float32r: fp32 rounded to 11
mantissa bits, full PE rate at N>=256; inputs pre-rounded on host so the
matmuls are exact w.r.t. the rounded operands):
  phase 1a: V = (x @ Wv + bv) -> DRAM bounce (frees SBUF for phase 1b)
  phase 1b: QT/KT pair tiles [128, S] = (Wq/k_pair_cols.T @ x.T + b)
  phase 2:  per (head, q-chunk of 512): scoresT[k,q] chunks via PE,
            additive causal mask (DVE), exp (ACT, scale=1/8) -> E^T fp32r,
            PV accumulation with a ones-column-augmented V giving both
            attn_outT [64, q] and the softmax row sums in one matmul chain,
            then normalize by reciprocal row sums (DVE) into pair-stacked
            outT tiles (odd heads shifted to partitions 64..127 via DMA).
  phase 3:  out_partial[s, :] = outT_pairs.T @ Wo_rows, DMA to DRAM.
"""

import numpy as np
from contextlib import ExitStack

import concourse.bass as bass
import concourse.tile as tile
import concourse.mybir as mybir
from concourse import bacc
from concourse.bass_utils import run_bass_kernel_spmd

F32 = mybir.dt.float32
F32R = mybir.dt.float32r
BF16 = mybir.dt.bfloat16
ActFn = mybir.ActivationFunctionType

import os
USE_BF16 = os.environ.get("ATTN_MM_DT", "fp32r") == "bf16"
MMDT = BF16 if USE_BF16 else F32R

B, S, D = 4, 2048, 1024
H, HD = 16, 64
NCORES = 8
HG = 512           # Wq/Wk/Wv columns (and Wo rows) per core
HPC = 8            # heads per core
P = 128
NDIN = D // P      # 8 contraction chunks for projections
NSC4 = S // 512    # 4 s-chunks of 512
NSC1 = S // P      # 16 s-chunks of 128
NPAIR = 4          # head pairs per core
NEG = -1.0e30


def _build_kernel(ctx: ExitStack, tc: tile.TileContext, io: dict):
    nc = tc.nc
    xt, wq, wk, wv, wo = io["xt"], io["wq"], io["wk"], io["wv"], io["wo"]
    bq, bk, bv, masks, out = io["bq"], io["bk"], io["bv"], io["masks"], io["out"]

    xt_r = xt.rearrange("(a p) s -> a p s", p=P)    # [8, 128, 2048]
    wq_r = wq.rearrange("(a p) n -> a p n", p=P)    # [8, 128, 512]
    wk_r = wk.rearrange("(a p) n -> a p n", p=P)
    wv_r = wv.rearrange("(a p) n -> a p n", p=P)
    wo_r = wo.rearrange("(a p) n -> a p n", p=P)    # [4, 128, 1024]
    bq_r = bq.rearrange("(pr p one) -> pr p one", p=P, one=1)  # [4, 128, 1]
    bk_r = bk.rearrange("(pr p one) -> pr p one", p=P, one=1)

    # ---- persistent pools (whole kernel) ----
    persist = ctx.enter_context(tc.tile_pool(name="persist", bufs=1))
    dram = ctx.enter_context(tc.tile_pool(name="dram", bufs=1, space="DRAM"))

    qt_sb = [persist.tile([P, S], MMDT, tag=f"qt{p_}", name=f"qt{p_}")
             for p_ in range(NPAIR)]
    kt_sb = [persist.tile([P, S], MMDT, tag=f"kt{p_}", name=f"kt{p_}")
             for p_ in range(NPAIR)]
    bq_sb = [persist.tile([P, 1], F32, tag=f"bq{p_}", name=f"bq{p_}")
             for p_ in range(NPAIR)]
    bk_sb = [persist.tile([P, 1], F32, tag=f"bk{p_}", name=f"bk{p_}")
             for p_ in range(NPAIR)]
    bv_sb = persist.tile([P, HG], F32, tag="bv", name="bv_sb")

    vb = dram.tile([NSC1, P, HG], MMDT, tag="vb", name="vb")
    sums_d = dram.tile([HPC, NSC4, 512], F32, tag="sums", name="sums_d")

    for p_ in range(NPAIR):
        nc.sync.dma_start(out=bq_sb[p_][:], in_=bq_r[p_])
        nc.sync.dma_start(out=bk_sb[p_][:], in_=bk_r[p_])
    # broadcast bv [512] across 128 partitions
    bv_b = bass.AP(tensor=bv.tensor, offset=bv.offset,
                   ap=[[0, P]] + [list(a) for a in bv.ap])
    nc.gpsimd.dma_start(out=bv_sb[:], in_=bv_b)

    # ---- phase 1: projections ----
    with tc.tile_pool(name="p1", bufs=1) as p1, \
         tc.tile_pool(name="p1w", bufs=8) as p1w, \
         tc.tile_pool(name="p1tmp", bufs=3) as p1tmp, \
         tc.tile_pool(name="ps1", bufs=4, space="PSUM") as ps1:

        xt_sb = [p1.tile([P, S], MMDT, tag=f"xt{a}", name=f"xt{a}")
                 for a in range(NDIN)]
        for a in range(NDIN):
            nc.sync.dma_start(out=xt_sb[a][:], in_=xt_r[a])

        # --- 1a: V -> DRAM bounce ---
        wv_sb = [p1w.tile([P, HG], MMDT, tag="w", name=f"wv{a}")
                 for a in range(NDIN)]
        for a in range(NDIN):
            nc.sync.dma_start(out=wv_sb[a][:], in_=wv_r[a])
        for sc in range(NSC1):
            ps = ps1.tile([P, HG], F32, tag="ps", name=f"vps{sc}")
            for a in range(NDIN):
                nc.tensor.matmul(ps[:], xt_sb[a][:, sc * P:(sc + 1) * P],
                                 wv_sb[a][:], start=(a == 0), stop=(a == NDIN - 1))
            vtmp = p1tmp.tile([P, HG], MMDT, tag="vtmp", name=f"vtmp{sc}")
            nc.vector.tensor_add(vtmp[:], ps[:], bv_sb[:])
            nc.sync.dma_start(out=vb[sc], in_=vtmp[:])

        # --- 1b: QT / KT pair tiles ---
        for (w_r, b_sb, dst) in ((wq_r, bq_sb, qt_sb), (wk_r, bk_sb, kt_sb)):
            w_sb = [p1w.tile([P, HG], MMDT, tag="w", name=f"w{a}")
                    for a in range(NDIN)]
            for a in range(NDIN):
                nc.sync.dma_start(out=w_sb[a][:], in_=w_r[a])
            for p_ in range(NPAIR):
                for sc in range(NSC4):
                    ps = ps1.tile([P, 512], F32, tag="ps", name=f"qkps{p_}_{sc}")
                    for a in range(NDIN):
                        nc.tensor.matmul(
                            ps[:], w_sb[a][:, p_ * P:(p_ + 1) * P],
                            xt_sb[a][:, sc * 512:(sc + 1) * 512],
                            start=(a == 0), stop=(a == NDIN - 1))
                    nc.scalar.activation(dst[p_][:, sc * 512:(sc + 1) * 512],
                                         ps[:], ActFn.Identity, bias=b_sb[p_][:])

    # load masks into SBUF [128, 4, 512]
    mask_t = persist.tile([P, 4, 512], F32, tag="maskt", name="mask_t")
    for v in range(4):
        nc.sync.dma_start(out=mask_t[:, v, :], in_=masks[v])

    # ---- phase 2: attention ----
    with tc.tile_pool(name="p23", bufs=1) as p23:
        v_sb = [p23.tile([P, HPC, 65], MMDT, tag=f"v{kb}", name=f"v{kb}")
                for kb in range(NSC1)]
        for kb in range(NSC1):
            nc.sync.dma_start(
                out=v_sb[kb][:, :, 0:64],
                in_=vb[kb].rearrange("p (h d) -> p h d", h=HPC))
            if USE_BF16:
                nc.vector.memset(v_sb[kb][:, :, 64:65], 1.0)
            else:
                nc.vector.memset(v_sb[kb][:, :, 64:65].bitcast(F32), 1.0)
        outt_sb = [p23.tile([P, S], MMDT, tag=f"ot{p_}", name=f"outt{p_}")
                   for p_ in range(NPAIR)]

        with tc.tile_pool(name="p2e", bufs=6) as p2e, \
             tc.tile_pool(name="p2r", bufs=3) as p2r, \
             tc.tile_pool(name="ps2s", bufs=3, space="PSUM") as ps2s, \
             tc.tile_pool(name="ps2o", bufs=2, space="PSUM") as ps2o:

            for h in range(HPC):
                pair, hoff = h // 2, (h % 2) * 64
                for qc in range(NSC4):
                    nkb = 4 * qc + 4
                    ot_ps = ps2o.tile([65, 512], F32, tag="ot",
                                      name=f"otps{h}_{qc}")
                    for kb in range(nkb):
                        # causal trim: diag-block chunks only need columns
                        # q >= kb*128, i.e. [off:512] of this 512-wide chunk
                        off = max(0, (kb - 4 * qc) * P)
                        nw = 512 - off
                        sc_ps = ps2s.tile([P, 512], F32, tag="sc",
                                          name=f"scps{h}_{qc}_{kb}")
                        nc.tensor.matmul(
                            sc_ps[:, 0:nw],
                            kt_sb[pair][hoff:hoff + 64, kb * P:(kb + 1) * P],
                            qt_sb[pair][hoff:hoff + 64,
                                        qc * 512 + off:(qc + 1) * 512],
                            start=True, stop=True)
                        if kb >= 4 * qc:
                            # triangular mask on the leading 128 cols (q == k)
                            nc.vector.tensor_add(sc_ps[:, 0:P], sc_ps[:, 0:P],
                                                 mask_t[:, 0, 0:P])
                        e_sb = p2e.tile([P, 512], MMDT, tag="e",
                                        name=f"e{h}_{qc}_{kb}")
                        nc.scalar.activation(e_sb[:, 0:nw], sc_ps[:, 0:nw],
                                             ActFn.Exp, scale=0.125)
                        nc.tensor.matmul(ot_ps[:, off:512],
                                         v_sb[kb][:, h, 0:65],
                                         e_sb[:, 0:nw], start=(kb == 0),
                                         stop=(kb == nkb - 1),
                                         skip_group_check=True)
                    # normalize: broadcast row sums (partition 64 of ot_ps)
                    # via DRAM, then approx-reciprocal + multiply on DVE
                    srow = p2r.tile([P, 512], F32, tag="srow",
                                    name=f"srow{h}_{qc}")
                    nc.scalar.copy(srow[64:65, :], ot_ps[64:65, :])
                    nc.sync.dma_start(out=sums_d[h, qc], in_=srow[64:65, :])
                    sb_ = p2r.tile([64, 512], F32, tag="sb", name=f"sb{h}_{qc}")
                    s_ap = sums_d[h, qc]
                    sb_b = bass.AP(tensor=s_ap.tensor, offset=s_ap.offset,
                                   ap=[[0, 64]] + [list(a) for a in s_ap.ap])
                    nc.gpsimd.dma_start(out=sb_[:], in_=sb_b)
                    rb = p2r.tile([64, 512], F32, tag="rb", name=f"rb{h}_{qc}")
                    nc.vector.reciprocal_approx_fast(rb[:], sb_[:])
                    if h % 2 == 0:
                        nc.vector.tensor_mul(
                            outt_sb[pair][0:64, qc * 512:(qc + 1) * 512],
                            ot_ps[0:64, :], rb[:])
                    else:
                        osc = p2r.tile([64, 512], MMDT, tag="osc",
                                       name=f"osc{h}_{qc}")
                        nc.vector.tensor_mul(osc[:], ot_ps[0:64, :], rb[:])
                        nc.sync.dma_start(
                            out=outt_sb[pair][64:128, qc * 512:(qc + 1) * 512],
                            in_=osc[:])

        # ---- phase 3: output projection ----
        with tc.tile_pool(name="p3", bufs=1) as p3, \
             tc.tile_pool(name="p3b", bufs=3) as p3b, \
             tc.tile_pool(name="ps3", bufs=4, space="PSUM") as ps3:
            wo_sb = [p3.tile([P, D], MMDT, tag=f"wo{a}", name=f"wo{a}")
                     for a in range(NPAIR)]
            for a in range(NPAIR):
                nc.sync.dma_start(out=wo_sb[a][:], in_=wo_r[a])
            for sc in range(NSC1):
                for nh in range(2):
                    ps = ps3.tile([P, 512], F32, tag="ps",
                                  name=f"ops{sc}_{nh}")
                    for a in range(NPAIR):
                        nc.tensor.matmul(
                            ps[:], outt_sb[a][:, sc * P:(sc + 1) * P],
                            wo_sb[a][:, nh * 512:(nh + 1) * 512],
                            start=(a == 0), stop=(a == NPAIR - 1))
                    ob = p3b.tile([P, 512], F32, tag="ob",
                                  name=f"ob{sc}_{nh}")
                    nc.scalar.copy(ob[:], ps[:])
                    nc.sync.dma_start(
                        out=out[sc * P:(sc + 1) * P, nh * 512:(nh + 1) * 512],
                        in_=ob[:])


def build_bass():
    nc = bacc.Bacc()
    io = {
        "xt": nc.dram_tensor("xt", [D, S], MMDT, kind="ExternalInput").ap(),
        "wq": nc.dram_tensor("wq", [D, HG], MMDT, kind="ExternalInput").ap(),
        "wk": nc.dram_tensor("wk", [D, HG], MMDT, kind="ExternalInput").ap(),
        "wv": nc.dram_tensor("wv", [D, HG], MMDT, kind="ExternalInput").ap(),
        "wo": nc.dram_tensor("wo", [HG, D], MMDT, kind="ExternalInput").ap(),
        "bq": nc.dram_tensor("bq", [HG], F32, kind="ExternalInput").ap(),
        "bk": nc.dram_tensor("bk", [HG], F32, kind="ExternalInput").ap(),
        "bv": nc.dram_tensor("bv", [HG], F32, kind="ExternalInput").ap(),
        "masks": nc.dram_tensor("masks", [4, P, 512], F32,
                                kind="ExternalInput").ap(),
        "out": nc.dram_tensor("out", [S, D], F32, kind="ExternalOutput").ap(),
    }
    with tile.TileContext(nc) as tc, ExitStack() as ctx:
        _build_kernel(ctx, tc, io)
    nc.compile()
    return nc


def round_fp32r(a: np.ndarray) -> np.ndarray:
    """Round fp32 to the 11-mantissa-bit float32r grid (nearest even)."""
    a = np.ascontiguousarray(a, dtype=np.float32)
    u = a.view(np.uint32)
    lm = np.uint32((1 << 12) - 1)
    half = np.uint32(1 << 11)
    low = u & lm
    hi = u & ~lm
    inc = (low > half) | ((low == half) &
                          (((hi >> np.uint32(12)) & np.uint32(1)).astype(bool)))
    return (hi + inc.astype(np.uint32) * np.uint32(1 << 12)).view(np.float32)


def to_mm(a: np.ndarray) -> np.ndarray:
    """Convert host fp32 to the matmul operand dtype's bit-exact grid."""
    if USE_BF16:
        import ml_dtypes
        return np.ascontiguousarray(a, dtype=np.float32).astype(ml_dtypes.bfloat16)
    return round_fp32r(a)


_NC_CACHE = {}


def _get_nc():
    if "nc" not in _NC_CACHE:
        _NC_CACHE["nc"] = build_bass()
    return _NC_CACHE["nc"]


def _make_masks() -> np.ndarray:
    k = np.arange(P)[:, None]
    c = np.arange(512)[None, :]
    m = np.zeros((4, P, 512), dtype=np.float32)
    for v in range(4):
        m[v] = np.where((c - 128 * v) >= k, 0.0, NEG)
    return m


def kernel(x, Wq, bq, Wk, bk, Wv, bv, Wo, bo, _trace=False):
    nc = _get_nc()
    x = np.asarray(x, dtype=np.float32)
    Wq, Wk, Wv, Wo = (np.asarray(w, dtype=np.float32) for w in (Wq, Wk, Wv, Wo))
    bq, bk, bv, bo = (np.asarray(b_, dtype=np.float32) for b_ in (bq, bk, bv, bo))
    masks = _make_masks()

    in_maps = []
    for c in range(NCORES):
        b_, hg = c // 2, c % 2
        cols = slice(hg * HG, (hg + 1) * HG)
        in_maps.append({
            "xt": to_mm(x[b_].T),
            "wq": to_mm(Wq[:, cols]),
            "wk": to_mm(Wk[:, cols]),
            "wv": to_mm(Wv[:, cols]),
            "wo": to_mm(Wo[cols, :]),
            "bq": np.ascontiguousarray(bq[cols]),
            "bk": np.ascontiguousarray(bk[cols]),
            "bv": np.ascontiguousarray(bv[cols]),
            "masks": masks,
        })
    res = run_bass_kernel_spmd(nc, in_maps, list(range(NCORES)), trace=_trace)
    out = np.empty((B, S, D), dtype=np.float32)
    for b_ in range(B):
        out[b_] = res.results[2 * b_]["out"] + res.results[2 * b_ + 1]["out"]
    out += bo[None, None, :]
    if _trace:
        return out, res
    return out


# revision 12
# speedup vs baseline: 1.4085x; 1.1237x over previous
"""Causal self-attention TRN2 Bass kernel.

Problem: B=4, S=2048, D=1024, H=16 heads, head_dim=64, fp32.
Sharding (8 cores): core c handles batch b = c//2 and head-half hg = c%2
(heads hg*8 .. hg*8+7, i.e. columns hg*512..+512 of Wq/Wk/Wv and rows
hg*512..+512 of Wo).  Each core produces a partial [S, D] output; the host
sums the two head-half partials per batch and adds bo.

On-device pipeline per core (all matmuls in float32r: fp32 rounded to 11
mantissa bits, full PE rate at N>=256; inputs pre-rounded on host so the
matmuls are exact w.r.t. the rounded operands):
  phase 1a: V = (x @ Wv + bv) -> DRAM bounce (frees SBUF for phase 1b)
  phase 1b: QT/KT pair tiles [128, S] = (Wq/k_pair_cols.T @ x.T + b)
  phase 2:  per (head, q-chunk of 512): scoresT[k,q] chunks via PE,
            additive causal mask (DVE), exp (ACT, scale=1/8) -> E^T fp32r,
            PV accumulation with a ones-column-augmented V giving both
            attn_outT [64, q] and the softmax row sums in one matmul chain,
            then normalize by reciprocal row sums (DVE) into pair-stacked
            outT tiles (odd heads shifted to partitions 64..127 via DMA).
  phase 3:  out_partial[s, :] = outT_pairs.T @ Wo_rows, DMA to DRAM.
"""

import numpy as np
from contextlib import ExitStack

import concourse.bass as bass
import concourse.tile as tile
import concourse.mybir as mybir
from concourse import bacc
from concourse.bass_utils import run_bass_kernel_spmd

F32 = mybir.dt.float32
F32R = mybir.dt.float32r
BF16 = mybir.dt.bfloat16
ActFn = mybir.ActivationFunctionType

import os
USE_BF16 = os.environ.get("ATTN_MM_DT", "fp32r") == "bf16"
MMDT = BF16 if USE_BF16 else F32R

B, S, D = 4, 2048, 1024
H, HD = 16, 64
NCORES = 8
HG = 512           # Wq/Wk/Wv columns (and Wo rows) per core
HPC = 8            # heads per core
P = 128
NDIN = D // P      # 8 contraction chunks for projections
NSC4 = S // 512    # 4 s-chunks of 512
NSC1 = S // P      # 16 s-chunks of 128
NPAIR = 4          # head pairs per core
NEG = -1.0e30


def _build_kernel(ctx: ExitStack, tc: tile.TileContext, io: dict):
    nc = tc.nc
    xt, wq, wk, wv, wo = io["xt"], io["wq"], io["wk"], io["wv"], io["wo"]
    bq, bk, bv, masks, out = io["bq"], io["bk"], io["bv"], io["masks"], io["out"]

    xt_r = xt.rearrange("(a p) s -> a p s", p=P)    # [8, 128, 2048]
    wq_r = wq.rearrange("(a p) n -> a p n", p=P)    # [8, 128, 512]
    wk_r = wk.rearrange("(a p) n -> a p n", p=P)
    wv_r = wv.rearrange("(a p) n -> a p n", p=P)
    wo_r = wo.rearrange("(a p) n -> a p n", p=P)    # [4, 128, 1024]
    bq_r = bq.rearrange("(pr p one) -> pr p one", p=P, one=1)  # [4, 128, 1]
    bk_r = bk.rearrange("(pr p one) -> pr p one", p=P, one=1)

    # ---- persistent pools (whole kernel) ----
    persist = ctx.enter_context(tc.tile_pool(name="persist", bufs=1))
    dram = ctx.enter_context(tc.tile_pool(name="dram", bufs=1, space="DRAM"))

    qt_sb = [persist.tile([P, S], MMDT, tag=f"qt{p_}", name=f"qt{p_}")
             for p_ in range(NPAIR)]
    kt_sb = [persist.tile([P, S], MMDT, tag=f"kt{p_}", name=f"kt{p_}")
             for p_ in range(NPAIR)]
    bq_sb = [persist.tile([P, 1], F32, tag=f"bq{p_}", name=f"bq{p_}")
             for p_ in range(NPAIR)]
    bk_sb = [persist.tile([P, 1], F32, tag=f"bk{p_}", name=f"bk{p_}")
             for p_ in range(NPAIR)]
    bv_sb = persist.tile([P, HG], F32, tag="bv", name="bv_sb")

    vb = dram.tile([NSC1, P, HG], MMDT, tag="vb", name="vb")
    sums_d = dram.tile([HPC, NSC4, 512], F32, tag="sums", name="sums_d")

    for p_ in range(NPAIR):
        nc.sync.dma_start(out=bq_sb[p_][:], in_=bq_r[p_])
        nc.sync.dma_start(out=bk_sb[p_][:], in_=bk_r[p_])
    # broadcast bv [512] across 128 partitions
    bv_b = bass.AP(tensor=bv.tensor, offset=bv.offset,
                   ap=[[0, P]] + [list(a) for a in bv.ap])
    nc.gpsimd.dma_start(out=bv_sb[:], in_=bv_b)

    # ---- phase 1: projections ----
    with tc.tile_pool(name="p1", bufs=1) as p1, \
         tc.tile_pool(name="p1w", bufs=8) as p1w, \
         tc.tile_pool(name="p1tmp", bufs=3) as p1tmp, \
         tc.tile_pool(name="ps1", bufs=4, space="PSUM") as ps1:

        xt_sb = [p1.tile([P, S], MMDT, tag=f"xt{a}", name=f"xt{a}")
                 for a in range(NDIN)]
        # chunked loads (s-major) so the first projection matmuls can
        # start as soon as the first 512-column slab of every din chunk lands
        for c in range(NSC4):
            for a in range(NDIN):
                nc.sync.dma_start(out=xt_sb[a][:, c * 512:(c + 1) * 512],
                                  in_=xt_r[a][:, c * 512:(c + 1) * 512])

        # --- 1a: V -> DRAM bounce ---
        wv_sb = [p1w.tile([P, HG], MMDT, tag="w", name=f"wv{a}")
                 for a in range(NDIN)]
        for a in range(NDIN):
            nc.sync.dma_start(out=wv_sb[a][:], in_=wv_r[a])
        for sc in range(NSC1):
            ps = ps1.tile([P, HG], F32, tag="ps", name=f"vps{sc}")
            for a in range(NDIN):
                nc.tensor.matmul(ps[:], xt_sb[a][:, sc * P:(sc + 1) * P],
                                 wv_sb[a][:], start=(a == 0), stop=(a == NDIN - 1))
            vtmp = p1tmp.tile([P, HG], MMDT, tag="vtmp", name=f"vtmp{sc}")
            nc.vector.tensor_add(vtmp[:], ps[:], bv_sb[:])
            nc.sync.dma_start(out=vb[sc], in_=vtmp[:])

        # --- 1b: QT / KT pair tiles ---
        for (w_r, b_sb, dst) in ((wq_r, bq_sb, qt_sb), (wk_r, bk_sb, kt_sb)):
            w_sb = [p1w.tile([P, HG], MMDT, tag="w", name=f"w{a}")
                    for a in range(NDIN)]
            for a in range(NDIN):
                nc.sync.dma_start(out=w_sb[a][:], in_=w_r[a])
            for p_ in range(NPAIR):
                for sc in range(NSC4):
                    ps = ps1.tile([P, 512], F32, tag="ps", name=f"qkps{p_}_{sc}")
                    for a in range(NDIN):
                        nc.tensor.matmul(
                            ps[:], w_sb[a][:, p_ * P:(p_ + 1) * P],
                            xt_sb[a][:, sc * 512:(sc + 1) * 512],
                            start=(a == 0), stop=(a == NDIN - 1))
                    nc.scalar.activation(dst[p_][:, sc * 512:(sc + 1) * 512],
                                         ps[:], ActFn.Identity, bias=b_sb[p_][:])

    # load masks into SBUF [128, 4, 512]
    mask_t = persist.tile([P, 4, 512], F32, tag="maskt", name="mask_t")
    for v in range(4):
        nc.sync.dma_start(out=mask_t[:, v, :], in_=masks[v])

    # ---- phase 2: attention ----
    with tc.tile_pool(name="p23", bufs=1) as p23:
        v_sb = [p23.tile([P, HPC, 65], MMDT, tag=f"v{kb}", name=f"v{kb}")
                for kb in range(NSC1)]
        for kb in range(NSC1):
            nc.sync.dma_start(
                out=v_sb[kb][:, :, 0:64],
                in_=vb[kb].rearrange("p (h d) -> p h d", h=HPC))
            if USE_BF16:
                nc.vector.memset(v_sb[kb][:, :, 64:65], 1.0)
            else:
                nc.vector.memset(v_sb[kb][:, :, 64:65].bitcast(F32), 1.0)
        outt_sb = [p23.tile([P, S], MMDT, tag=f"ot{p_}", name=f"outt{p_}")
                   for p_ in range(NPAIR)]

        with tc.tile_pool(name="p2e", bufs=8) as p2e, \
             tc.tile_pool(name="p2r", bufs=3) as p2r, \
             tc.tile_pool(name="ps2s", bufs=4, space="PSUM") as ps2s, \
             tc.tile_pool(name="ps2o", bufs=2, space="PSUM") as ps2o:

            for h in range(HPC):
                pair, hoff = h // 2, (h % 2) * 64
                for qc in range(NSC4):
                    nkb = 4 * qc + 4
                    ot_ps = ps2o.tile([65, 512], F32, tag="ot",
                                      name=f"otps{h}_{qc}")

                    # software-pipelined: emit PV one iteration behind the
                    # scores/exp chain so the PE's in-order stream never
                    # head-of-line blocks on the exp (ACT) result
                    def emit_scores(kb):
                        off = max(0, (kb - 4 * qc) * P)
                        nw = 512 - off
                        sc_ps = ps2s.tile([P, 512], F32, tag="sc",
                                          name=f"scps{h}_{qc}_{kb}")
                        nc.tensor.matmul(
                            sc_ps[:, 0:nw],
                            kt_sb[pair][hoff:hoff + 64, kb * P:(kb + 1) * P],
                            qt_sb[pair][hoff:hoff + 64,
                                        qc * 512 + off:(qc + 1) * 512],
                            start=True, stop=True)
                        if kb >= 4 * qc:
                            # triangular mask on the leading 128 cols (q == k)
                            nc.vector.tensor_add(sc_ps[:, 0:P], sc_ps[:, 0:P],
                                                 mask_t[:, 0, 0:P])
                        e_sb = p2e.tile([P, 512], MMDT, tag="e",
                                        name=f"e{h}_{qc}_{kb}")
                        nc.scalar.activation(e_sb[:, 0:nw], sc_ps[:, 0:nw],
                                             ActFn.Exp, scale=0.125)
                        return e_sb, off, nw

                    def emit_pv(kb, e_sb, off, nw):
                        nc.tensor.matmul(ot_ps[:, off:512],
                                         v_sb[kb][:, h, 0:65],
                                         e_sb[:, 0:nw], start=(kb == 0),
                                         stop=(kb == nkb - 1),
                                         skip_group_check=True)

                    pending = None
                    for kb in range(nkb):
                        e_info = emit_scores(kb)
                        if pending is not None:
                            emit_pv(*pending)
                        pending = (kb,) + e_info
                    emit_pv(*pending)
                    # normalize: broadcast row sums (partition 64 of ot_ps)
                    # via DRAM, then approx-reciprocal + multiply on DVE
                    srow = p2r.tile([P, 512], F32, tag="srow",
                                    name=f"srow{h}_{qc}")
                    nc.scalar.copy(srow[64:65, :], ot_ps[64:65, :])
                    nc.sync.dma_start(out=sums_d[h, qc], in_=srow[64:65, :])
                    sb_ = p2r.tile([64, 512], F32, tag="sb", name=f"sb{h}_{qc}")
                    s_ap = sums_d[h, qc]
                    sb_b = bass.AP(tensor=s_ap.tensor, offset=s_ap.offset,
                                   ap=[[0, 64]] + [list(a) for a in s_ap.ap])
                    nc.gpsimd.dma_start(out=sb_[:], in_=sb_b)
                    rb = p2r.tile([64, 512], F32, tag="rb", name=f"rb{h}_{qc}")
                    nc.vector.reciprocal_approx_fast(rb[:], sb_[:])
                    if h % 2 == 0:
                        nc.vector.tensor_mul(
                            outt_sb[pair][0:64, qc * 512:(qc + 1) * 512],
                            ot_ps[0:64, :], rb[:])
                    else:
                        osc = p2r.tile([64, 512], MMDT, tag="osc",
                                       name=f"osc{h}_{qc}")
                        nc.vector.tensor_mul(osc[:], ot_ps[0:64, :], rb[:])
                        nc.sync.dma_start(
                            out=outt_sb[pair][64:128, qc * 512:(qc + 1) * 512],
                            in_=osc[:])

        # ---- phase 3: output projection ----
        with tc.tile_pool(name="p3", bufs=1) as p3, \
             tc.tile_pool(name="p3b", bufs=3) as p3b, \
             tc.tile_pool(name="ps3", bufs=4, space="PSUM") as ps3:
            wo_sb = [p3.tile([P, D], MMDT, tag=f"wo{a}", name=f"wo{a}")
                     for a in range(NPAIR)]
            for a in range(NPAIR):
                nc.sync.dma_start(out=wo_sb[a][:], in_=wo_r[a])
            for sc in range(NSC1):
                for nh in range(2):
                    ps = ps3.tile([P, 512], F32, tag="ps",
                                  name=f"ops{sc}_{nh}")
                    for a in range(NPAIR):
                        nc.tensor.matmul(
                            ps[:], outt_sb[a][:, sc * P:(sc + 1) * P],
                            wo_sb[a][:, nh * 512:(nh + 1) * 512],
                            start=(a == 0), stop=(a == NPAIR - 1))
                    ob = p3b.tile([P, 512], F32, tag="ob",
                                  name=f"ob{sc}_{nh}")
                    nc.scalar.copy(ob[:], ps[:])
                    nc.sync.dma_start(
                        out=out[sc * P:(sc + 1) * P, nh * 512:(nh + 1) * 512],
                        in_=ob[:])


def build_bass():
    nc = bacc.Bacc()
    io = {
        "xt": nc.dram_tensor("xt", [D, S], MMDT, kind="ExternalInput").ap(),
        "wq": nc.dram_tensor("wq", [D, HG], MMDT, kind="ExternalInput").ap(),
        "wk": nc.dram_tensor("wk", [D, HG], MMDT, kind="ExternalInput").ap(),
        "wv": nc.dram_tensor("wv", [D, HG], MMDT, kind="ExternalInput").ap(),
        "wo": nc.dram_tensor("wo", [HG, D], MMDT, kind="ExternalInput").ap(),
        "bq": nc.dram_tensor("bq", [HG], F32, kind="ExternalInput").ap(),
        "bk": nc.dram_tensor("bk", [HG], F32, kind="ExternalInput").ap(),
        "bv": nc.dram_tensor("bv", [HG], F32, kind="ExternalInput").ap(),
        "masks": nc.dram_tensor("masks", [4, P, 512], F32,
                                kind="ExternalInput").ap(),
        "out": nc.dram_tensor("out", [S, D], F32, kind="ExternalOutput").ap(),
    }
    with tile.TileContext(nc) as tc, ExitStack() as ctx:
        _build_kernel(ctx, tc, io)
    nc.compile()
    return nc


def round_fp32r(a: np.ndarray) -> np.ndarray:
    """Round fp32 to the 11-mantissa-bit float32r grid (nearest even)."""
    a = np.ascontiguousarray(a, dtype=np.float32)
    u = a.view(np.uint32)
    lm = np.uint32((1 << 12) - 1)
    half = np.uint32(1 << 11)
    low = u & lm
    hi = u & ~lm
    inc = (low > half) | ((low == half) &
                          (((hi >> np.uint32(12)) & np.uint32(1)).astype(bool)))
    return (hi + inc.astype(np.uint32) * np.uint32(1 << 12)).view(np.float32)


def to_mm(a: np.ndarray) -> np.ndarray:
    """Convert host fp32 to the matmul operand dtype's bit-exact grid."""
    if USE_BF16:
        import ml_dtypes
        return np.ascontiguousarray(a, dtype=np.float32).astype(ml_dtypes.bfloat16)
    return round_fp32r(a)


_NC_CACHE = {}


def _get_nc():
    if "nc" not in _NC_CACHE:
        _NC_CACHE["nc"] = build_bass()
    return _NC_CACHE["nc"]


def _make_masks() -> np.ndarray:
    k = np.arange(P)[:, None]
    c = np.arange(512)[None, :]
    m = np.zeros((4, P, 512), dtype=np.float32)
    for v in range(4):
        m[v] = np.where((c - 128 * v) >= k, 0.0, NEG)
    return m


def kernel(x, Wq, bq, Wk, bk, Wv, bv, Wo, bo, _trace=False):
    nc = _get_nc()
    x = np.asarray(x, dtype=np.float32)
    Wq, Wk, Wv, Wo = (np.asarray(w, dtype=np.float32) for w in (Wq, Wk, Wv, Wo))
    bq, bk, bv, bo = (np.asarray(b_, dtype=np.float32) for b_ in (bq, bk, bv, bo))
    masks = _make_masks()

    in_maps = []
    for c in range(NCORES):
        b_, hg = c // 2, c % 2
        cols = slice(hg * HG, (hg + 1) * HG)
        in_maps.append({
            "xt": to_mm(x[b_].T),
            "wq": to_mm(Wq[:, cols]),
            "wk": to_mm(Wk[:, cols]),
            "wv": to_mm(Wv[:, cols]),
            "wo": to_mm(Wo[cols, :]),
            "bq": np.ascontiguousarray(bq[cols]),
            "bk": np.ascontiguousarray(bk[cols]),
            "bv": np.ascontiguousarray(bv[cols]),
            "masks": masks,
        })
    res = run_bass_kernel_spmd(nc, in_maps, list(range(NCORES)), trace=_trace)
    out = np.empty((B, S, D), dtype=np.float32)
    for b_ in range(B):
        out[b_] = res.results[2 * b_]["out"] + res.results[2 * b_ + 1]["out"]
    out += bo[None, None, :]
    if _trace:
        return out, res
    return out
